# revision 2
# baseline (speedup 1.0000x reference)
"""Trainium2 Bass kernel for nn_GroupLinear: channel-shuffled grouped MLP.

Computes, for x [4096, 16384]:
    h = relu(einsum('bgi,gio->bgo', x[:, perm1].reshape(B,128,128), W1) + b1)
    h = relu(einsum('bgi,gio->bgo', h.reshape(B,8192)[:, perm2].reshape(B,128,64), W2) + b2)
    y = sigmoid(((h.reshape(B,4096) @ W3 + b3) @ W4 + b4) @ W5 + b5)

Sharding: data-parallel over batch across 8 cores (512 rows/core); weights
replicated. The dense head has no intermediate activations, so W3@W4@W5 is
collapsed on the host into a single [4096,1] vector (exact algebra).
Permutations are runtime inputs but are baked into per-group gather index
tables at build time (the kernel is compiled inside kernel()).

Per-core pipeline:
  A) transpose x shard via PE into DRAM scratch xT [16384, NB]
  B) GL1: per group, indirect-DMA row-gather of perm1 channels -> matmul
     vs W1[g] (pairs packed into one PSUM bank via col tile_position) ->
     relu+bias -> plain write h1 [8192, NB]
  C) GL2: gather 128 perm2 rows (2 groups) per indirect DMA, block-diag W2
     pair matmuls -> quad-packed h2 [128, NB] tiles resident in SBUF
  D) head: 32 accumulating matmuls vs w345 chunks -> sigmoid -> y [1, NB]
"""

import hashlib

import numpy as np

import concourse.bass as bass
import concourse.mybir as mybir
import concourse.tile as tile
from concourse import bacc, bass_utils
from concourse.bass import IndirectOffsetOnAxis
from concourse.masks import make_identity

G = 128          # groups
C1 = G * 128     # 16384 input channels
C2 = G * 64      # 8192 channels after GL1
C3 = G * 32      # 4096 channels after GL2
N_CORES = 8
B_FULL = 4096

F32 = mybir.dt.float32
I32 = mybir.dt.int32


def build_nc(nb: int, b345: float, cblk: int = 1024):
    """Build the per-core Bass program for batch-per-core nb."""
    assert nb % 128 == 0
    nbt = nb // 128           # batch subtiles of 128
    ncb = C1 // cblk          # channel blocks in phase A
    ncc = cblk // 128         # 128-channel chunks per block

    nc = bacc.Bacc("TRN2", debug=False)
    xs = nc.dram_tensor("xs", [nb, C1], F32, kind="ExternalInput")
    w1 = nc.dram_tensor("w1", [128, G * 64], F32, kind="ExternalInput")
    w2 = nc.dram_tensor("w2", [128, 64 * 64], F32, kind="ExternalInput")
    w345 = nc.dram_tensor("w345", [128, 32], F32, kind="ExternalInput")
    b1p = nc.dram_tensor("b1p", [128, 64], F32, kind="ExternalInput")
    b2q = nc.dram_tensor("b2q", [128, 32], F32, kind="ExternalInput")
    p1i = nc.dram_tensor("p1i", [128, 128], I32, kind="ExternalInput")
    p2i = nc.dram_tensor("p2i", [128, 64], I32, kind="ExternalInput")
    y = nc.dram_tensor("y", [1, nb], F32, kind="ExternalOutput")

    relu_t = mybir.ActivationFunctionType.Relu
    sigm_t = mybir.ActivationFunctionType.Sigmoid

    with tile.TileContext(nc) as tc:
        with (
            tc.tile_pool(name="const", bufs=1) as cpool,
            tc.tile_pool(name="h2p", bufs=1) as h2pool,
            tc.tile_pool(name="work", bufs=2) as pool,
            tc.tile_pool(name="psum", bufs=2, space="PSUM") as psum,
            tc.tile_pool(name="dram", bufs=1, space="DRAM") as dpool,
        ):
            # ---- constants / weights preload ----
            w1s = cpool.tile([128, G * 64], F32)
            nc.sync.dma_start(w1s[:], w1.ap())
            w2s = cpool.tile([128, 64 * 64], F32)
            nc.sync.dma_start(w2s[:], w2.ap())
            w345s = cpool.tile([128, 32], F32)
            nc.sync.dma_start(w345s[:], w345.ap())
            b1s = cpool.tile([128, 64], F32)
            nc.sync.dma_start(b1s[:], b1p.ap())
            b2s = cpool.tile([128, 32], F32)
            nc.sync.dma_start(b2s[:], b2q.ap())
            p1s = cpool.tile([128, 128], I32)
            nc.sync.dma_start(p1s[:], p1i.ap())
            p2s = cpool.tile([128, 64], I32)
            nc.sync.dma_start(p2s[:], p2i.ap())
            ident = cpool.tile([128, 128], F32)
            make_identity(nc, ident[:])

            xTd = dpool.tile([C1, nb], F32)
            h1d = dpool.tile([C2, nb], F32)

            # ---- phase A: transpose x -> xT ----
            for cb in range(ncb):
                xn = []
                for bt in range(nbt):
                    t = pool.tile([128, cblk], F32, tag=f"xn{bt}", name=f"xn{bt}")
                    nc.sync.dma_start(
                        t[:], xs.ap()[bt * 128:(bt + 1) * 128,
                                      cb * cblk:(cb + 1) * cblk])
                    xn.append(t)
                for cc in range(ncc):
                    ch0 = cb * cblk + cc * 128
                    ps_a = psum.tile([128, nb], F32, tag="ps_a")
                    for bt in range(nbt):
                        nc.tensor.transpose(
                            ps_a[:, bt * 128:(bt + 1) * 128],
                            xn[bt][:, cc * 128:(cc + 1) * 128],
                            ident[:])
                    xtc = pool.tile([128, nb], F32, tag="xtc", bufs=4)
                    nc.vector.tensor_copy(xtc[:], ps_a[:])
                    nc.sync.dma_start(xTd[ch0:ch0 + 128, :], xtc[:])

            # ---- phase B: GL1 ----
            for k in range(G // 2):          # pairs of groups
                ps_b = psum.tile([128, nb], F32, tag="ps_b")
                for half in range(2):
                    g = 2 * k + half
                    rhs1 = pool.tile([128, nb], F32, tag="rhs1", bufs=4)
                    nc.gpsimd.indirect_dma_start(
                        out=rhs1[:], out_offset=None, in_=xTd[:],
                        in_offset=IndirectOffsetOnAxis(ap=p1s[:, g:g + 1], axis=0))
                    nc.tensor.matmul(
                        ps_b[64 * half:64 * (half + 1), :],
                        lhsT=w1s[:, g * 64:(g + 1) * 64], rhs=rhs1[:],
                        start=True, stop=True,
                        tile_position=(0, 64 * half) if half else None)
                h1t = pool.tile([128, nb], F32, tag="h1t", bufs=3)
                nc.scalar.activation(h1t[:], ps_b[:], relu_t, bias=b1s[:, k:k + 1])
                nc.sync.dma_start(h1d[k * 128:(k + 1) * 128, :], h1t[:])

            # ---- phase C: GL2 ----
            h2 = []
            for j in range(32):              # quads of groups
                ps_c = psum.tile([128, nb], F32, tag="ps_c")
                for half in range(2):
                    q = 2 * j + half         # pair index (gather of 2 groups)
                    rhs2 = pool.tile([128, nb], F32, tag="rhs2", bufs=4)
                    nc.gpsimd.indirect_dma_start(
                        out=rhs2[:], out_offset=None, in_=h1d[:],
                        in_offset=IndirectOffsetOnAxis(ap=p2s[:, q:q + 1], axis=0))
                    nc.tensor.matmul(
                        ps_c[64 * half:64 * (half + 1), :],
                        lhsT=w2s[:, q * 64:(q + 1) * 64], rhs=rhs2[:],
                        start=True, stop=True,
                        tile_position=(0, 64 * half) if half else None)
                h2t = h2pool.tile([128, nb], F32, tag=f"h2_{j}", name=f"h2_{j}")
                nc.scalar.activation(h2t[:], ps_c[:], relu_t, bias=b2s[:, j:j + 1])
                h2.append(h2t)

            # ---- phase D: head ----
            ps_d = psum.tile([1, nb], F32, tag="ps_d", bufs=1)
            for t in range(32):
                nc.tensor.matmul(ps_d[:], lhsT=w345s[:, t:t + 1], rhs=h2[t][:],
                                 start=(t == 0), stop=(t == 31))
            yt = pool.tile([1, nb], F32, tag="yt", bufs=1)
            nc.scalar.activation(yt[:], ps_d[:], sigm_t, bias=float(b345))
            nc.sync.dma_start(y.ap(), yt[:])

    nc.compile()
    return nc


def prep_host(perm1, perm2, W1, b1, W2, b2, W3, b3, W4, b4, W5, b5):
    """Host-side layout prep of weights / index tables (replicated per core)."""
    w1h = np.ascontiguousarray(
        W1.astype(np.float32).transpose(1, 0, 2).reshape(128, G * 64))
    w2h = np.zeros((128, 64 * 64), dtype=np.float32)
    for q in range(64):
        w2h[0:64, q * 64:q * 64 + 32] = W2[2 * q]
        w2h[64:128, q * 64 + 32:(q + 1) * 64] = W2[2 * q + 1]
    wv = (W3.astype(np.float64) @ W4.astype(np.float64) @ W5.astype(np.float64))
    w345h = np.ascontiguousarray(
        wv.astype(np.float32).reshape(32, 128).T)
    b345 = float(
        (b3.astype(np.float64) @ W4.astype(np.float64) @ W5.astype(np.float64)
         + b4.astype(np.float64) @ W5.astype(np.float64)
         + b5.astype(np.float64)).reshape(()))
    b1h = np.ascontiguousarray(
        b1.astype(np.float32).reshape(64, 128).T)   # col k = [b1[2k]; b1[2k+1]]
    b2h = np.ascontiguousarray(
        b2.astype(np.float32).reshape(32, 128).T)   # col j = b2[4j:4j+4] stacked
    p1h = np.ascontiguousarray(perm1.astype(np.int32).reshape(128, 128).T)
    p2h = np.ascontiguousarray(perm2.astype(np.int32).reshape(64, 128).T)
    return {"w1": w1h, "w2": w2h, "w345": w345h, "b1p": b1h, "b2q": b2h,
            "p1i": p1h, "p2i": p2h}, b345


_NC_CACHE: dict = {}


def get_nc(nb: int, b345: float, key_bytes: bytes):
    key = (nb, hashlib.sha256(key_bytes + np.float64(b345).tobytes()).hexdigest())
    if key not in _NC_CACHE:
        _NC_CACHE[key] = build_nc(nb, b345)
    return _NC_CACHE[key]


def kernel(x, perm1, perm2, W1, b1, W2, b2, W3, b3, W4, b4, W5, b5):
    x = np.asarray(x)
    consts, b345 = prep_host(np.asarray(perm1), np.asarray(perm2),
                             np.asarray(W1), np.asarray(b1), np.asarray(W2),
                             np.asarray(b2), np.asarray(W3), np.asarray(b3),
                             np.asarray(W4), np.asarray(b4), np.asarray(W5),
                             np.asarray(b5))
    nb = x.shape[0] // N_CORES
    key_bytes = np.asarray(perm1).tobytes() + np.asarray(perm2).tobytes()
    nc = get_nc(nb, b345, key_bytes)
    in_maps = []
    for c in range(N_CORES):
        m = dict(consts)
        m["xs"] = np.ascontiguousarray(x[c * nb:(c + 1) * nb].astype(np.float32))
        in_maps.append(m)
    res = bass_utils.run_bass_kernel_spmd(nc, in_maps, core_ids=list(range(N_CORES)))
    out = np.concatenate([res.results[c]["y"].reshape(nb) for c in range(N_CORES)])
    return out.reshape(-1, 1).astype(np.float32)


# revision 10
# speedup vs baseline: 11.7003x; 11.7003x over previous
"""Trainium2 Bass kernel for nn_GroupLinear: channel-shuffled grouped MLP.

Computes, for x [4096, 16384]:
    h = relu(einsum('bgi,gio->bgo', x[:, perm1].reshape(B,128,128), W1) + b1)
    h = relu(einsum('bgi,gio->bgo', h.reshape(B,8192)[:, perm2].reshape(B,128,64), W2) + b2)
    y = sigmoid(((h.reshape(B,4096) @ W3 + b3) @ W4 + b4) @ W5 + b5)

Sharding: data-parallel over batch across 8 cores (512 rows/core); weights
replicated. The dense head has no intermediate activations, so W3@W4@W5 is
collapsed on the host into a single [4096,1] vector (exact algebra).
Permutations are runtime inputs but are baked into per-group gather index
tables at build time (the kernel is compiled inside kernel()).

Per-core pipeline:
  A) transpose x shard via PE into DRAM scratch xT [16384, NB]
  B) GL1: per group, indirect-DMA row-gather of perm1 channels -> matmul
     vs W1[g] (pairs packed into one PSUM bank via col tile_position) ->
     relu+bias -> plain write h1 [8192, NB]
  C) GL2: gather 128 perm2 rows (2 groups) per indirect DMA, block-diag W2
     pair matmuls -> quad-packed h2 [128, NB] tiles resident in SBUF
  D) head: 32 accumulating matmuls vs w345 chunks -> sigmoid -> y [1, NB]
"""

import hashlib

import numpy as np

import concourse.bass as bass
import concourse.mybir as mybir
import concourse.tile as tile
from concourse import bacc, bass_utils
from concourse.bass import IndirectOffsetOnAxis
from concourse.masks import make_identity

G = 128          # groups
C1 = G * 128     # 16384 input channels
C2 = G * 64      # 8192 channels after GL1
C3 = G * 32      # 4096 channels after GL2
N_CORES = 8
B_FULL = 4096

F32 = mybir.dt.float32
I32 = mybir.dt.int32


def build_nc(nb: int, b345: float, cblk: int = 1024, reps: int = 1,
             mid_bf16: int = 0, ablate_gathers: int = 0):
    """Build the per-core Bass program for batch-per-core nb.

    reps > 1 repeats the whole pipeline inside one NEFF (for timing:
    device time per rep = (T_reps - T_1) / (reps - 1), cancelling the
    fixed per-execution dispatch overhead).
    """
    assert nb % 128 == 0
    nbt = nb // 128           # batch subtiles of 128
    ncb = C1 // cblk          # channel blocks in phase A
    ncc = cblk // 128         # 128-channel chunks per block

    MID = mybir.dt.bfloat16 if mid_bf16 else F32
    nc = bacc.Bacc("TRN2", debug=False)
    xs = nc.dram_tensor("xs", [nb, C1], F32, kind="ExternalInput")
    w1 = nc.dram_tensor("w1", [128, G * 64], MID, kind="ExternalInput")
    w2 = nc.dram_tensor("w2", [128, 64 * 64], MID, kind="ExternalInput")
    w345 = nc.dram_tensor("w345", [128, 32], F32, kind="ExternalInput")
    b1p = nc.dram_tensor("b1p", [128, 64], F32, kind="ExternalInput")
    b2q = nc.dram_tensor("b2q", [128, 32], F32, kind="ExternalInput")
    p1i = nc.dram_tensor("p1i", [128, 128], I32, kind="ExternalInput")
    p2i = nc.dram_tensor("p2i", [128, 64], I32, kind="ExternalInput")
    y = nc.dram_tensor("y", [1, nb], F32, kind="ExternalOutput")

    relu_t = mybir.ActivationFunctionType.Relu
    sigm_t = mybir.ActivationFunctionType.Sigmoid

    with tile.TileContext(nc) as tc:
        with (
            tc.tile_pool(name="const", bufs=1) as cpool,
            tc.tile_pool(name="h2p", bufs=1) as h2pool,
            tc.tile_pool(name="work", bufs=2) as pool,
            tc.tile_pool(name="psum", bufs=2, space="PSUM") as psum,
            tc.tile_pool(name="dram", bufs=1, space="DRAM") as dpool,
        ):
            # ---- constants / weights preload ----
            w1s = cpool.tile([128, G * 64], MID)
            nc.sync.dma_start(w1s[:], w1.ap())
            w2s = cpool.tile([128, 64 * 64], MID)
            nc.sync.dma_start(w2s[:], w2.ap())
            w345s = cpool.tile([128, 32], F32)
            nc.sync.dma_start(w345s[:], w345.ap())
            b1s = cpool.tile([128, 64], F32)
            nc.sync.dma_start(b1s[:], b1p.ap())
            b2s = cpool.tile([128, 32], F32)
            nc.sync.dma_start(b2s[:], b2q.ap())
            p1s = cpool.tile([128, 128], I32)
            nc.sync.dma_start(p1s[:], p1i.ap())
            p2s = cpool.tile([128, 64], I32)
            nc.sync.dma_start(p2s[:], p2i.ap())
            ident = cpool.tile([128, 128], F32)
            make_identity(nc, ident[:])

            xTd = dpool.tile([C1, nb], MID)
            h1d = dpool.tile([C2, nb], MID)

            for _rep in range(reps):
                run_pipeline(nc, tc, pool, psum, h2pool, xs, y, xTd, h1d,
                             w1s, w2s, w345s, b1s, b2s, p1s, p2s, ident,
                             nb, nbt, ncb, ncc, cblk, b345, relu_t, sigm_t, MID,
                             ablate_gathers)

    nc.compile()
    return nc


def run_pipeline(nc, tc, pool, psum, h2pool, xs, y, xTd, h1d,
                 w1s, w2s, w345s, b1s, b2s, p1s, p2s, ident,
                 nb, nbt, ncb, ncc, cblk, b345, relu_t, sigm_t, MID,
                 ablate_gathers=0):
    if True:
        if True:
            # ---- phase A: transpose x -> xT ----
            for cb in range(ncb):
                xn = []
                for bt in range(nbt):
                    t = pool.tile([128, cblk], F32, tag=f"xn{bt}", name=f"xn{bt}")
                    nc.sync.dma_start(
                        t[:], xs.ap()[bt * 128:(bt + 1) * 128,
                                      cb * cblk:(cb + 1) * cblk])
                    xn.append(t)
                for cc in range(ncc):
                    ch0 = cb * cblk + cc * 128
                    ps_a = psum.tile([128, nb], F32, tag="ps_a")
                    for bt in range(nbt):
                        nc.tensor.transpose(
                            ps_a[:, bt * 128:(bt + 1) * 128],
                            xn[bt][:, cc * 128:(cc + 1) * 128],
                            ident[:])
                    xtc = pool.tile([128, nb], MID, tag="xtc", bufs=4)
                    nc.vector.tensor_copy(xtc[:], ps_a[:])
                    nc.sync.dma_start(xTd[ch0:ch0 + 128, :], xtc[:])

            # ---- phase B: GL1 ----
            for k in range(G // 2):          # pairs of groups
                ps_b = psum.tile([128, nb], F32, tag="ps_b")
                for half in range(2):
                    g = 2 * k + half
                    rhs1 = pool.tile([128, nb], MID, tag="rhs1", bufs=4)
                    nc.gpsimd.indirect_dma_start(
                        out=rhs1[:], out_offset=None, in_=xTd[:],
                        in_offset=IndirectOffsetOnAxis(ap=p1s[:, g:g + 1],
                                                       axis=0))
                    nc.tensor.matmul(
                        ps_b[64 * half:64 * (half + 1), :],
                        lhsT=w1s[:, g * 64:(g + 1) * 64], rhs=rhs1[:],
                        start=True, stop=True,
                        tile_position=(0, 64 * half) if half else None)
                h1t = pool.tile([128, nb], MID, tag="h1t", bufs=3)
                nc.scalar.activation(h1t[:], ps_b[:], relu_t, bias=b1s[:, k:k + 1])
                nc.sync.dma_start(h1d[k * 128:(k + 1) * 128, :], h1t[:])

            # ---- phase C: GL2 ----
            h2 = []
            for j in range(32):              # quads of groups
                ps_c = psum.tile([128, nb], F32, tag="ps_c")
                for half in range(2):
                    q = 2 * j + half         # pair index (gather of 2 groups)
                    rhs2 = pool.tile([128, nb], MID, tag="rhs2", bufs=4)
                    if ablate_gathers:
                        nc.sync.dma_start(rhs2[:], h1d[q * 128:(q + 1) * 128, :])
                    else:
                        nc.gpsimd.indirect_dma_start(
                            out=rhs2[:], out_offset=None, in_=h1d[:],
                            in_offset=IndirectOffsetOnAxis(ap=p2s[:, q:q + 1],
                                                           axis=0))
                    nc.tensor.matmul(
                        ps_c[64 * half:64 * (half + 1), :],
                        lhsT=w2s[:, q * 64:(q + 1) * 64], rhs=rhs2[:],
                        start=True, stop=True,
                        tile_position=(0, 64 * half) if half else None)
                h2t = h2pool.tile([128, nb], F32, tag=f"h2_{j}", name=f"h2_{j}")
                nc.scalar.activation(h2t[:], ps_c[:], relu_t, bias=b2s[:, j:j + 1])
                h2.append(h2t)

            # ---- phase D: head ----
            ps_d = psum.tile([1, nb], F32, tag="ps_d", bufs=1)
            for t in range(32):
                nc.tensor.matmul(ps_d[:], lhsT=w345s[:, t:t + 1], rhs=h2[t][:],
                                 start=(t == 0), stop=(t == 31))
            yt = pool.tile([1, nb], F32, tag="yt", bufs=1)
            nc.scalar.activation(yt[:], ps_d[:], sigm_t, bias=float(b345))
            nc.sync.dma_start(y.ap(), yt[:])


def prep_host(perm1, perm2, W1, b1, W2, b2, W3, b3, W4, b4, W5, b5,
              mid_bf16=0):
    """Host-side layout prep of weights / index tables (replicated per core)."""
    import ml_dtypes
    wdt = ml_dtypes.bfloat16 if mid_bf16 else np.float32
    w1h = np.ascontiguousarray(
        W1.astype(np.float32).transpose(1, 0, 2).reshape(128, G * 64)).astype(wdt)
    w2h = np.zeros((128, 64 * 64), dtype=wdt)
    for q in range(64):
        w2h[0:64, q * 64:q * 64 + 32] = W2[2 * q].astype(wdt)
        w2h[64:128, q * 64 + 32:(q + 1) * 64] = W2[2 * q + 1].astype(wdt)
    wv = (W3.astype(np.float64) @ W4.astype(np.float64) @ W5.astype(np.float64))
    w345h = np.ascontiguousarray(
        wv.astype(np.float32).reshape(32, 128).T)
    b345 = float(
        (b3.astype(np.float64) @ W4.astype(np.float64) @ W5.astype(np.float64)
         + b4.astype(np.float64) @ W5.astype(np.float64)
         + b5.astype(np.float64)).reshape(()))
    b1h = np.ascontiguousarray(
        b1.astype(np.float32).reshape(64, 128).T)   # col k = [b1[2k]; b1[2k+1]]
    b2h = np.ascontiguousarray(
        b2.astype(np.float32).reshape(32, 128).T)   # col j = b2[4j:4j+4] stacked
    p1h = np.ascontiguousarray(perm1.astype(np.int32).reshape(128, 128).T)
    p2h = np.ascontiguousarray(perm2.astype(np.int32).reshape(64, 128).T)
    return {"w1": w1h, "w2": w2h, "w345": w345h, "b1p": b1h, "b2q": b2h,
            "p1i": p1h, "p2i": p2h}, b345


_NC_CACHE: dict = {}

# Default precision for the DRAM-staged intermediates (xT, h1) and W1/W2.
# 0 = float32 everywhere.  1 = bf16 staging (halves staging traffic; adds
# ~1e-3 relative error vs the f32 reference, still well within tolerance).
MID_BF16 = 1


def get_nc(nb: int, b345: float, key_bytes: bytes, mid_bf16: int = MID_BF16):
    key = (nb, mid_bf16,
           hashlib.sha256(key_bytes + np.float64(b345).tobytes()).hexdigest())
    if key not in _NC_CACHE:
        _NC_CACHE[key] = build_nc(nb, b345, mid_bf16=mid_bf16)
    return _NC_CACHE[key]


def kernel(x, perm1, perm2, W1, b1, W2, b2, W3, b3, W4, b4, W5, b5):
    x = np.asarray(x)
    consts, b345 = prep_host(np.asarray(perm1), np.asarray(perm2),
                             np.asarray(W1), np.asarray(b1), np.asarray(W2),
                             np.asarray(b2), np.asarray(W3), np.asarray(b3),
                             np.asarray(W4), np.asarray(b4), np.asarray(W5),
                             np.asarray(b5), mid_bf16=MID_BF16)
    nb = x.shape[0] // N_CORES
    key_bytes = np.asarray(perm1).tobytes() + np.asarray(perm2).tobytes()
    nc = get_nc(nb, b345, key_bytes)
    in_maps = []
    for c in range(N_CORES):
        m = dict(consts)
        m["xs"] = np.ascontiguousarray(x[c * nb:(c + 1) * nb].astype(np.float32))
        in_maps.append(m)
    res = bass_utils.run_bass_kernel_spmd(nc, in_maps, core_ids=list(range(N_CORES)))
    out = np.concatenate([res.results[c]["y"].reshape(nb) for c in range(N_CORES)])
    return out.reshape(-1, 1).astype(np.float32)
